# revision 20
# baseline (speedup 1.0000x reference)
"""Bass/Trainium2 SPMD kernel for nn_GCLMessage (GNN message passing).

Strategy (8 NeuronCores):
  - Edges sharded contiguously: 20000/core (padded to 20480).
  - Each core: LayerNorm on its 1280-node shard -> projects pa=xh@A, pb=xh@B
    (A/B = first-layer weight blocks for the xh[ii]/xh[jj] operands),
    AllGather -> full pa/pb tables in DRAM.
  - Edge pipeline per 512-edge macro-tile, feature-major ("transposed") layout:
      m1T[h,e] = sum_d C[d,h]*wT[d,e] (+ pa[ii].T + pb[jj].T via PE transpose-
      accumulate) -> silu -> m2T -> silu -> attT -> silu -> m_ijT = m2T*att
      y[e,:] = m_ij@Wo + bo -> silu -> + w  -> edgeh rows.
      m_ij also written (natural layout) to a DRAM scratch for aggregation.
  - Aggregation: per 128-node window, CSR-gather m_ij rows (host-sorted edge
    positions, zero-row padded) + one-hot matmuls accumulate agg[node,h] in
    PSUM -> per-core partial agg[10240,128].
  - ReduceScatter(add) over 8 cores -> per-core agg shard [1280,128].
  - Node MLP on the shard; outputs xh_out shard + edgeh shard; host reassembles.

Host passes weight shards both natural [20480,416] and transposed [416,20480]
so the device never transposes the big stream.
"""

import numpy as np

import concourse.bacc as bacc_mod
import concourse.mybir as mybir
from concourse.tile import TileContext
from concourse.bass_utils import run_bass_kernel_spmd

F32 = mybir.dt.float32
F32R = mybir.dt.float32r
I16 = mybir.dt.int16
AF = mybir.ActivationFunctionType
ALU = mybir.AluOpType

HID = 128
NRAD = 32
EDIM = 3 * HID + NRAD          # 416
N_NODES = 10000
N_EDGES = 160000
NCORES = 8
NODES_PAD = 10240
NSHARD = NODES_PAD // NCORES   # 1280
NWTILE = NSHARD // 128         # 10 node tiles per shard
ESHARD = N_EDGES // NCORES     # 20000
MACRO = 512                    # edges per macro-tile
NMACRO = 40                    # ceil(20000/512) -> 20480
EPAD = NMACRO * MACRO          # 20480
ECH = MACRO // 128             # e-chunks per macro-tile (4)
WINDOWS = NODES_PAD // 128     # 80 aggregation windows
WPAD = 512                     # padded gather positions per window
WCH = WPAD // 128              # chunks per window (4)
ZROW = EPAD                    # zero sentinel row in m_ij scratch
DCH = [128, 128, 128, 32]      # contraction chunks of EDIM=416
EPS = 1e-5

MM_FAST = True                 # f32r (tf32-ish) for the big matmuls


def _r(ap):
    """View an f32 AP as f32r for fast matmul (no data movement)."""
    return ap.bitcast(F32R)


def build_nc(fast=MM_FAST):
    nc = bacc_mod.Bacc("TRN2", target_bir_lowering=False, num_devices=NCORES)
    MDT = F32R if fast else F32   # dtype tag for fast-matmul operands

    di = lambda n, s, dt=F32: nc.dram_tensor(n, s, dt, kind="ExternalInput")
    # --- inputs (per core) ---
    w_nat = di("w_nat", [EPAD, EDIM])
    w_tr = di("w_tr", [EDIM, EPAD], MDT)
    x_shard = di("x_shard", [NSHARD, HID])
    Cw = di("Cw", [EDIM, HID], MDT)       # We1[2H:,:]
    Aw = di("Aw", [HID, HID], MDT)        # We1[:H,:]
    Bw = di("Bw", [HID, HID], MDT)        # We1[H:2H,:]
    We2 = di("We2", [HID, HID], MDT)
    Wa = di("Wa", [HID, 1], MDT)
    Wo = di("Wo", [HID, EDIM], MDT)
    Wn1 = di("Wn1", [2 * HID, HID], MDT)
    Wn2 = di("Wn2", [HID, HID], MDT)
    be1 = di("be1", [HID, 1])
    be2 = di("be2", [HID, 1])
    ba = di("ba", [1, 1])
    bo_row = di("bo_row", [1, EDIM], MDT)
    bn1 = di("bn1", [HID, 1])
    bn2 = di("bn2", [HID, 1])
    eps_c = di("eps_c", [HID, 1])
    g_bc = di("g_bc", [128, HID])
    b_bc = di("b_bc", [128, HID])
    eye = di("eye", [128, 128])
    ones_col = di("ones_col", [1, 128], MDT)
    iota_row = di("iota_row", [128, 128])
    inv_cnt = di("inv_cnt", [128, NWTILE])
    nrel = di("nrel", [128, WINDOWS * WCH])
    idx_ii = nc.dram_tensor("idx_ii", [128, EPAD // 16], I16, kind="ExternalInput")
    idx_jj = nc.dram_tensor("idx_jj", [128, EPAD // 16], I16, kind="ExternalInput")
    idx_csr = nc.dram_tensor("idx_csr", [128, WINDOWS * WPAD // 16], I16,
                             kind="ExternalInput")
    # --- outputs (per core) ---
    xh_out = nc.dram_tensor("xh_out", [NSHARD, HID], F32, kind="ExternalOutput")
    edgeh = nc.dram_tensor("edgeh", [EPAD, EDIM], F32, kind="ExternalOutput")

    def tmm(out, in_, identity, start, stop):
        """PE transpose with PSUM accumulation control."""
        nc.tensor.matmul(out, in_, identity, is_transpose=True,
                         start=start, stop=stop)

    with TileContext(nc) as tc:
        with (
            tc.tile_pool(name="const", bufs=1) as cp,
            tc.tile_pool(name="sb", bufs=2) as sb,
            tc.tile_pool(name="sb3", bufs=3) as sb3,
            tc.tile_pool(name="ps", bufs=1, space="PSUM") as psp,
            tc.tile_pool(name="ps2", bufs=2, space="PSUM") as psp2,
            tc.tile_pool(name="dram", bufs=1, space="DRAM") as dp,
        ):
            # ---------- constants to SBUF ----------
            def load_const(src, shape, dt=F32):
                t = cp.tile(shape, dt, tag="c_" + src.name)
                nc.sync.dma_start(out=t[:, :], in_=src[:, :])
                return t

            # C [416,128] as 4 partition-chunks [<=128, 128] in one tile
            tC = cp.tile([128, 4, HID], MDT)
            for k in range(4):
                nc.sync.dma_start(out=tC[0:DCH[k], k, :],
                                  in_=Cw[k * 128:k * 128 + DCH[k], :])
            tWn1 = cp.tile([128, 2, HID], MDT)
            for k in range(2):
                nc.sync.dma_start(out=tWn1[:, k, :],
                                  in_=Wn1[k * 128:(k + 1) * 128, :])
            tA = load_const(Aw, [HID, HID], MDT)
            tB = load_const(Bw, [HID, HID], MDT)
            tWe2 = load_const(We2, [HID, HID], MDT)
            tWa = load_const(Wa, [HID, 1], MDT)
            tWo = load_const(Wo, [HID, EDIM], MDT)
            tWn2 = load_const(Wn2, [HID, HID], MDT)
            tbe1 = load_const(be1, [HID, 1])
            tbe2 = load_const(be2, [HID, 1])
            tba = load_const(ba, [1, 1])
            tbo = load_const(bo_row, [1, EDIM], MDT)
            tbn1 = load_const(bn1, [HID, 1])
            tbn2 = load_const(bn2, [HID, 1])
            teps = load_const(eps_c, [HID, 1])
            tg = load_const(g_bc, [128, HID])
            tb_ = load_const(b_bc, [128, HID])
            teye = load_const(eye, [128, 128])
            teyeR = cp.tile([128, 128], MDT, tag="c_eyeR")
            nc.sync.dma_start(out=teyeR[:, :], in_=eye[:, :].bitcast(MDT))
            tones = load_const(ones_col, [1, 128], MDT)
            tiota = load_const(iota_row, [128, 128])
            ticnt = load_const(inv_cnt, [128, NWTILE])
            tnrel = load_const(nrel, [128, WINDOWS * WCH])
            tii = cp.tile([128, EPAD // 16], I16)
            nc.sync.dma_start(out=tii[:, :], in_=idx_ii[:, :])
            tjj = cp.tile([128, EPAD // 16], I16)
            nc.sync.dma_start(out=tjj[:, :], in_=idx_jj[:, :])
            tcsr = cp.tile([128, WINDOWS * WPAD // 16], I16)
            nc.sync.dma_start(out=tcsr[:, :], in_=idx_csr[:, :])

            # ---------- DRAM scratch ----------
            pa_my = dp.tile([NSHARD, HID], F32)
            pb_my = dp.tile([NSHARD, HID], F32)
            pa_full = dp.tile([NODES_PAD, HID], F32)
            pb_full = dp.tile([NODES_PAD, HID], F32)
            mij_scr = dp.tile([EPAD + 128, HID], F32)
            agg_full = dp.tile([NODES_PAD, HID], F32)
            agg_my = dp.tile([NSHARD, HID], F32)

            # ---------- phase 1: LayerNorm shard + projections ----------
            xh_sb = cp.tile([128, NWTILE, HID], F32)      # resident xh shard
            xhT = cp.tile([128, NWTILE * 128], MDT)       # resident xh^T shard
            for t in range(NWTILE):
                xt = sb.tile([128, HID], F32, tag="ln_x")
                nc.sync.dma_start(out=xt[:, :], in_=x_shard[t * 128:(t + 1) * 128, :])
                s = sb.tile([128, 1], F32, tag="ln_s")
                nc.vector.tensor_reduce(s[:, :], xt[:, :], axis=mybir.AxisListType.X,
                                        op=ALU.add)
                mu = sb.tile([128, 1], F32, tag="ln_mu")
                nc.vector.tensor_scalar_mul(mu[:, :], s[:, :], 1.0 / HID)
                xc = sb.tile([128, HID], F32, tag="ln_xc")
                nc.vector.tensor_scalar(xc[:, :], xt[:, :], mu[:, :], None,
                                        op0=ALU.subtract)
                sqt = sb.tile([128, HID], F32, tag="ln_sq")
                v = sb.tile([128, 1], F32, tag="ln_v")
                nc.scalar.activation(sqt[:, :], xc[:, :], AF.Square,
                                     accum_out=v[:, :])
                sd = sb.tile([128, 1], F32, tag="ln_sd")
                nc.scalar.activation(sd[:, :], v[:, :], AF.Sqrt,
                                     bias=teps[:, :], scale=1.0 / HID)
                rs = sb.tile([128, 1], F32, tag="ln_rs")
                nc.vector.reciprocal(rs[:, :], sd[:, :])
                xn = sb.tile([128, HID], F32, tag="ln_xn")
                nc.vector.tensor_scalar(xn[:, :], xc[:, :], rs[:, :], None,
                                        op0=ALU.mult)
                xng = sb.tile([128, HID], F32, tag="ln_xng")
                nc.vector.tensor_tensor(xng[:, :], xn[:, :], tg[:, :], op=ALU.mult)
                nc.vector.tensor_tensor(xh_sb[:, t, :], xng[:, :], tb_[:, :],
                                        op=ALU.add)
                # transpose into resident xhT
                pst = psp.tile([128, 128], F32, tag="p_mn")
                nc.tensor.transpose(pst[:, :], xh_sb[:, t, :], teye[:, :])
                nc.vector.tensor_copy(xhT[:, t * 128:(t + 1) * 128], pst[:, :])

            # paT/pbT = A.T @ xhT, B.T @ xhT ; then transpose back + store
            for (tbl, dst) in ((tA, pa_my), (tB, pb_my)):
                pT = sb.tile([128, NSHARD], F32, tag="projT")
                for f0 in range(0, NSHARD, 512):
                    fw = min(512, NSHARD - f0)
                    pp = psp.tile([128, 512], F32, tag="p_m1")
                    nc.tensor.matmul(pp[:, 0:fw], tbl[:, :],
                                     xhT[:, f0:f0 + fw],
                                     start=True, stop=True)
                    nc.vector.tensor_copy(pT[:, f0:f0 + fw], pp[:, 0:fw])
                for t in range(NWTILE):
                    psn = psp.tile([128, 128], F32, tag="p_mn")
                    nc.tensor.transpose(psn[:, :], pT[:, t * 128:(t + 1) * 128],
                                        teye[:, :])
                    nat = sb.tile([128, HID], F32, tag="projN")
                    nc.vector.tensor_copy(nat[:, :], psn[:, :])
                    nc.sync.dma_start(out=dst[t * 128:(t + 1) * 128, :],
                                      in_=nat[:, :])

            # AllGather shard tables -> full tables
            rg = [list(range(NCORES))]
            nc.gpsimd.collective_compute("AllGather", ALU.bypass,
                                         replica_groups=rg,
                                         ins=[pa_my.opt()], outs=[pa_full.opt()])
            nc.gpsimd.collective_compute("AllGather", ALU.bypass,
                                         replica_groups=rg,
                                         ins=[pb_my.opt()], outs=[pb_full.opt()])

            # zero sentinel rows of mij scratch
            zrow = cp.tile([128, HID], F32)
            nc.vector.memset(zrow[:, :], 0.0)
            nc.sync.dma_start(out=mij_scr[EPAD:EPAD + 128, :], in_=zrow[:, :])

            # ---------- phase 2: edge pipeline ----------
            for m in range(NMACRO):
                e0 = m * MACRO
                # transposed weight tile [416, 512] as 4 k-chunks in one tile
                wT = sb.tile([128, ECH, 512], MDT, tag="wT")
                for k in range(4):
                    d0 = k * 128
                    nc.sync.dma_start(
                        out=wT[0:DCH[k], k, :],
                        in_=w_tr[d0:d0 + DCH[k], e0:e0 + MACRO])
                # natural weight tile (for residual) [128, ECH, 416]
                wN = sb.tile([128, ECH, EDIM], F32, tag="wN")
                nc.sync.dma_start(
                    out=wN[:, :, :],
                    in_=w_nat[e0:e0 + MACRO, :].rearrange("(c p) d -> p c d", p=128))
                # gathers: pa[ii], pb[jj] -> [128, ECH, HID]
                pag = sb.tile([128, ECH, HID], F32, tag="pag")
                nc.gpsimd.dma_gather(
                    out_ap=pag[:, :, :], in_ap=pa_full[:, :],
                    idxs_ap=tii[:, e0 // 16:(e0 + MACRO) // 16],
                    num_idxs=MACRO, num_idxs_reg=MACRO, elem_size=HID)
                pbg = sb.tile([128, ECH, HID], F32, tag="pbg")
                nc.gpsimd.dma_gather(
                    out_ap=pbg[:, :, :], in_ap=pb_full[:, :],
                    idxs_ap=tjj[:, e0 // 16:(e0 + MACRO) // 16],
                    num_idxs=MACRO, num_idxs_reg=MACRO, elem_size=HID)

                # m1T = C.T @ wT (+ transposed gathers)
                pm1 = psp.tile([128, MACRO], F32, tag="p_m1")
                for k in range(4):
                    nc.tensor.matmul(pm1[:, :], tC[0:DCH[k], k, :],
                                     wT[0:DCH[k], k, :],
                                     start=(k == 0), stop=False)
                for c in range(ECH):
                    tmm(pm1[:, c * 128:(c + 1) * 128], pag[:, c, :], teye[:, :],
                        start=False, stop=False)
                    tmm(pm1[:, c * 128:(c + 1) * 128], pbg[:, c, :], teye[:, :],
                        start=False, stop=(c == ECH - 1))
                m1T = sb.tile([128, MACRO], MDT, tag="m1T")
                nc.scalar.activation(m1T[:, :], pm1[:, :], AF.Silu, bias=tbe1[:, :])

                # m2T
                pm2 = psp.tile([128, MACRO], F32, tag="p_m2")
                nc.tensor.matmul(pm2[:, :], tWe2[:, :], m1T[:, :],
                                 start=True, stop=True)
                m2T = sb.tile([128, MACRO], MDT, tag="m2T")
                nc.scalar.activation(m2T[:, :], pm2[:, :], AF.Silu, bias=tbe2[:, :])

                # attT [1, 512]
                pat = psp.tile([1, MACRO], F32, tag="p_att")
                nc.tensor.matmul(pat[:, :], tWa[:, :], m2T[:, :],
                                 start=True, stop=True)
                attT = sb.tile([1, MACRO], MDT, tag="attT")
                nc.scalar.activation(attT[:, :], pat[:, :], AF.Silu, bias=tba[:, :])

                # att broadcast to 128 partitions
                pab = psp.tile([128, MACRO], F32, tag="p_ab")
                nc.tensor.matmul(pab[:, :], tones[:, :], attT[:, :],
                                 start=True, stop=True)
                mijT = sb.tile([128, MACRO], MDT, tag="mijT")
                nc.vector.tensor_tensor(mijT[:, :], m2T[:, :], pab[:, :],
                                        op=ALU.mult)

                # m_ij natural -> scratch (for aggregation)
                pmn = psp.tile([128, MACRO], F32, tag="p_mn")
                for c in range(ECH):
                    tmm(pmn[:, c * 128:(c + 1) * 128].bitcast(MDT),
                        mijT[:, c * 128:(c + 1) * 128], teyeR[:, :],
                        start=(c == 0), stop=(c == ECH - 1))
                mijN = sb.tile([128, ECH, HID], F32, tag="mijN")
                nc.vector.tensor_copy(mijN[:, :, :],
                                      pmn[:, :].rearrange("p (c h) -> p c h", c=ECH))
                nc.sync.dma_start(
                    out=mij_scr[e0:e0 + MACRO, :].rearrange("(c p) d -> p c d", p=128),
                    in_=mijN[:, :, :])

                # edgeh = w + silu(m_ij @ Wo + bo), per e-chunk
                for c in range(ECH):
                    py = psp2.tile([128, EDIM], F32, tag="p_y")
                    nc.tensor.matmul(py[:, :], tones[:, :], tbo[:, :],
                                     start=True, stop=False)
                    nc.tensor.matmul(py[:, :],
                                     mijT[:, c * 128:(c + 1) * 128],
                                     tWo[:, :], start=False, stop=True)
                    t3 = sb3.tile([128, EDIM], F32, tag="t3")
                    nc.scalar.activation(t3[:, :], py[:, :], AF.Silu)
                    oe = sb3.tile([128, EDIM], F32, tag="oe")
                    nc.vector.tensor_tensor(oe[:, :], t3[:, :], wN[:, c, :],
                                            op=ALU.add)
                    nc.sync.dma_start(out=edgeh[e0 + c * 128:e0 + (c + 1) * 128, :],
                                      in_=oe[:, :])

            # ---------- phase 3: windowed aggregation ----------
            for w in range(WINDOWS):
                gat = sb.tile([128, WCH, HID], F32, tag="gat")
                i0 = w * WPAD // 16
                nc.gpsimd.dma_gather(
                    out_ap=gat[:, :, :], in_ap=mij_scr[:, :],
                    idxs_ap=tcsr[:, i0:i0 + WPAD // 16],
                    num_idxs=WPAD, num_idxs_reg=WPAD, elem_size=HID)
                pagg = psp.tile([128, HID], F32, tag="p_att")
                for c in range(WCH):
                    oh = sb.tile([128, 128], F32, tag="onehot")
                    nc.vector.tensor_scalar(oh[:, :], tiota[:, :],
                                            tnrel[:, w * WCH + c:w * WCH + c + 1],
                                            None, op0=ALU.is_equal)
                    nc.tensor.matmul(pagg[:, :], oh[:, :], gat[:, c, :],
                                     start=(c == 0), stop=(c == WCH - 1))
                aggt = sb.tile([128, HID], F32, tag="aggt")
                nc.vector.tensor_copy(aggt[:, :], pagg[:, :])
                nc.sync.dma_start(out=agg_full[w * 128:(w + 1) * 128, :],
                                  in_=aggt[:, :])

            # ---------- phase 4: ReduceScatter + node MLP ----------
            nc.gpsimd.collective_compute("ReduceScatter", ALU.add,
                                         replica_groups=rg,
                                         ins=[agg_full.opt()], outs=[agg_my.opt()])

            aggT = cp.tile([128, NSHARD], MDT)    # normalized agg, transposed
            for t in range(NWTILE):
                at = sb.tile([128, HID], F32, tag="agg_in")
                nc.sync.dma_start(out=at[:, :], in_=agg_my[t * 128:(t + 1) * 128, :])
                an = sb.tile([128, HID], F32, tag="agg_n")
                nc.vector.tensor_scalar(an[:, :], at[:, :],
                                        ticnt[:, t:t + 1], None, op0=ALU.mult)
                pst = psp.tile([128, 128], F32, tag="p_mn")
                nc.tensor.transpose(pst[:, :], an[:, :], teye[:, :])
                nc.vector.tensor_copy(aggT[:, t * 128:(t + 1) * 128], pst[:, :])

            n1T = cp.tile([128, NSHARD], MDT)
            for f0 in range(0, NSHARD, 512):
                fw = min(512, NSHARD - f0)
                pn1 = psp.tile([128, 512], F32, tag="p_m1")
                nc.tensor.matmul(pn1[:, 0:fw], tWn1[:, 0, :],
                                 xhT[:, f0:f0 + fw], start=True, stop=False)
                nc.tensor.matmul(pn1[:, 0:fw], tWn1[:, 1, :],
                                 aggT[:, f0:f0 + fw], start=False, stop=True)
                nc.scalar.activation(n1T[:, f0:f0 + fw], pn1[:, 0:fw], AF.Silu,
                                     bias=tbn1[:, :])
            n2T = cp.tile([128, NSHARD], F32)
            for f0 in range(0, NSHARD, 512):
                fw = min(512, NSHARD - f0)
                pn2 = psp.tile([128, 512], F32, tag="p_m2")
                nc.tensor.matmul(pn2[:, 0:fw], tWn2[:, :],
                                 n1T[:, f0:f0 + fw], start=True, stop=True)
                nc.scalar.activation(n2T[:, f0:f0 + fw], pn2[:, 0:fw], AF.Silu,
                                     bias=tbn2[:, :])
            for t in range(NWTILE):
                psn = psp.tile([128, 128], F32, tag="p_mn")
                nc.tensor.transpose(psn[:, :], n2T[:, t * 128:(t + 1) * 128],
                                    teye[:, :])
                xo = sb.tile([128, HID], F32, tag="xh_o")
                nc.vector.tensor_tensor(xo[:, :], xh_sb[:, t, :], psn[:, :],
                                        op=ALU.add)
                nc.sync.dma_start(out=xh_out[t * 128:(t + 1) * 128, :], in_=xo[:, :])

    nc.finalize()
    return nc


def _wrap_idx(ids, pad_to):
    """int array -> dma_gather idx layout [128, pad_to//16] int16."""
    a = np.full(pad_to, -1, np.int64)
    a[:len(ids)] = ids
    a = a.astype(np.int16).reshape(-1, 16).T          # [16, pad/16]
    return np.tile(a, (8, 1))                          # [128, pad/16]


_NC_CACHE = {}


def _get_nc():
    if "nc" not in _NC_CACHE:
        _NC_CACHE["nc"] = build_nc()
    return _NC_CACHE["nc"]


def kernel(x, weight, ln_g, ln_b, We1, be1, We2, be2, Wa, ba,
           Wn1, bn1, Wn2, bn2, Wo, bo, edge_index, _trace=False):
    x = np.asarray(x); weight = np.asarray(weight)
    edge_index = np.asarray(edge_index)
    ii_all, jj_all = edge_index[0], edge_index[1]

    cnt = np.bincount(ii_all, minlength=NODES_PAD).astype(np.float64)
    inv_cnt_full = (1.0 / np.where(cnt == 0, 1.0, cnt)).astype(np.float32)

    g_bc = np.tile(np.asarray(ln_g, np.float32)[None, :], (128, 1))
    b_bc = np.tile(np.asarray(ln_b, np.float32)[None, :], (128, 1))
    eye = np.eye(128, dtype=np.float32)
    ones_col = np.ones((1, 128), np.float32)
    iota_row = np.tile(np.arange(128, dtype=np.float32)[None, :], (128, 1))

    common = dict(
        Cw=np.ascontiguousarray(We1[2 * HID:, :], np.float32),
        Aw=np.ascontiguousarray(We1[:HID, :], np.float32),
        Bw=np.ascontiguousarray(We1[HID:2 * HID, :], np.float32),
        We2=np.asarray(We2, np.float32), Wa=np.asarray(Wa, np.float32),
        Wo=np.asarray(Wo, np.float32), Wn1=np.asarray(Wn1, np.float32),
        Wn2=np.asarray(Wn2, np.float32),
        be1=np.asarray(be1, np.float32).reshape(HID, 1),
        be2=np.asarray(be2, np.float32).reshape(HID, 1),
        ba=np.asarray(ba, np.float32).reshape(1, 1),
        bo_row=np.asarray(bo, np.float32).reshape(1, EDIM),
        bn1=np.asarray(bn1, np.float32).reshape(HID, 1),
        bn2=np.asarray(bn2, np.float32).reshape(HID, 1),
        eps_c=np.full((HID, 1), EPS, np.float32),
        g_bc=g_bc, b_bc=b_bc, eye=eye, ones_col=ones_col, iota_row=iota_row,
    )

    in_maps = []
    for c in range(NCORES):
        es, ee = c * ESHARD, (c + 1) * ESHARD
        ii = ii_all[es:ee].astype(np.int64)
        jj = jj_all[es:ee].astype(np.int64)

        w_shard = np.zeros((EPAD, EDIM), np.float32)
        w_shard[:ESHARD] = weight[es:ee]
        w_tr = np.ascontiguousarray(w_shard.T)

        ii_pad = np.full(EPAD, NODES_PAD - 1, np.int64)   # dummies -> dead node
        ii_pad[:ESHARD] = ii
        jj_pad = np.zeros(EPAD, np.int64)
        jj_pad[:ESHARD] = jj

        # CSR windows: positions of edges sorted by dest node, per 128-window
        order = np.argsort(ii, kind="stable")
        dst_w = ii[order] // 128
        csr = np.full((WINDOWS, WPAD), ZROW, np.int64)
        rel = np.full((WINDOWS, WCH * 128), -1.0, np.float32)
        starts = np.searchsorted(dst_w, np.arange(WINDOWS + 1))
        for w in range(WINDOWS):
            lo, hi = starts[w], starts[w + 1]
            n = hi - lo
            if n > WPAD:
                raise ValueError(f"window overflow: {n} > {WPAD}")
            csr[w, :n] = order[lo:hi]
            rel[w, :n] = (ii[order[lo:hi]] - w * 128).astype(np.float32)
        # nrel layout [128, WINDOWS*WCH]: column w*WCH+c, partition p
        nrel_arr = np.ascontiguousarray(
            rel.reshape(WINDOWS * WCH, 128).T)

        ns, ne = c * NSHARD, (c + 1) * NSHARD
        x_shard = np.zeros((NSHARD, HID), np.float32)
        real = min(ne, N_NODES) - ns
        if real > 0:
            x_shard[:real] = x[ns:ns + real]

        m = dict(common)
        m.update(
            w_nat=w_shard, w_tr=w_tr, x_shard=x_shard,
            inv_cnt=np.ascontiguousarray(
                inv_cnt_full[ns:ne].reshape(NWTILE, 128).T),
            nrel=nrel_arr,
            idx_ii=_wrap_idx(ii_pad, EPAD),
            idx_jj=_wrap_idx(jj_pad, EPAD),
            idx_csr=_wrap_idx(csr.reshape(-1), WINDOWS * WPAD),
        )
        in_maps.append(m)

    nc = _get_nc()
    res = run_bass_kernel_spmd(nc, in_maps, core_ids=list(range(NCORES)),
                               trace=_trace)
    if _trace:
        _NC_CACHE["last_result"] = res

    xh_out = np.concatenate([r["xh_out"] for r in res.results], 0)[:N_NODES]
    edgeh = np.concatenate([r["edgeh"][:ESHARD] for r in res.results], 0)
    return xh_out, edgeh


if __name__ == "__main__":
    import jax
    jax.device_put(np.ones(4, np.float32), jax.devices()[0]) * 2
    nc = build_nc()
    print("built ok")
